# revision 1
# baseline (speedup 1.0000x reference)
"""Unfold/im2col kernel for Trainium2 (Bass/Tile), 8-core data parallel.

Problem: x [4, 64, 224, 224] f32 -> out [4, 576, 49729] f32 where
out[b, (c*3+kh)*3+kw, oh*223+ow] = pad(x,1)[b, c, oh+kh, ow+kw]
(3x3 kernel, pad 1, stride 1, dilation 1, oh=ow=223).

Sharding: 8 cores = (batch 4) x (channel half 2). Each core handles
32 channels -> [288, 49729] independently; outputs concatenate on the
channel axis (channel-major row layout makes halves contiguous).

The input is zero-padded host-side to [32, 226, 226] per core, so the
device kernel is pure DMA. All 32 padded images live in two SBUF tiles
(padded rows 0..127 / 128..225 on partitions, channels side by side in
the free dim), each filled by ONE load DMA. Each (kh, kw) window is
then written by one DMA per tile half per 16-channel block via a 3D
access pattern (window-row x channel x 223). Big stores issue on
gpsimd (SWDGE): its model-queue DMAs are spread across all 16 SDMA
engines (~230 GB/s at this 892 B descriptor size), whereas the HWDGE
dynamic rings feed a single SDMA engine (~15-28 GB/s) and only carry
the tiny split-remainder chunks. Measured ~308 us/core on TRN2
(roofline for 57 MB out + 6.5 MB in at ~358 GB/s HBM is ~180 us; the
892 B descriptor processing rate of the SDMA engines is the binding
limit).
"""

from contextlib import ExitStack

import numpy as np

import concourse.bass as bass
import concourse.tile as tile
from concourse import mybir
from concourse.ap import AP
from concourse.bass_utils import run_bass_kernel_spmd

B, C, IH, IW = 4, 64, 224, 224
N_CORES = 8
CPC = C // 2          # channels per core: 32
PH = IH + 2           # padded height/width: 226
OH = IH - 1           # output spatial: 223
OSZ = OH * OH         # 49729
NROW = CPC * 9        # 288 output rows per core
ROWS0 = 128           # padded rows 0..127 in tile0
ROWS1 = PH - ROWS0    # padded rows 128..225 in tile1 (98)
FREE = CPC * PH       # free dim elements per tile: 7232
PIMG = PH * PH        # padded image elements: 51076

_NC_CACHE = {}


def build_nc() -> bass.Bass:
    nc = bass.Bass()
    x = nc.declare_dram_parameter("xp", [CPC, PH, PH], mybir.dt.float32, isOutput=False)
    out = nc.declare_dram_parameter("out", [NROW, OSZ], mybir.dt.float32, isOutput=True)
    xb = x[:, :, :]
    ob = out[:, :]

    with tile.TileContext(nc) as tc:
        with ExitStack() as ctx:
            pool = ctx.enter_context(tc.tile_pool(name="img", bufs=1))
            t0 = pool.tile([ROWS0, FREE], mybir.dt.float32, name="t0", tag="t0")[:, :]
            t1 = pool.tile([ROWS1, FREE], mybir.dt.float32, name="t1", tag="t1")[:, :]

            # Two loads: tile partition p, free col c*226+w  <-  xp[c, p(+128), w]
            src0 = AP(xb.tensor, xb.offset,
                      [[PH, ROWS0], [PIMG, CPC], [1, PH]])
            dst0 = AP(t0.tensor, t0.offset,
                      [[FREE, ROWS0], [PH, CPC], [1, PH]])
            nc.gpsimd.dma_start(out=dst0, in_=src0)
            src1 = AP(xb.tensor, xb.offset + ROWS0 * PH,
                      [[PH, ROWS1], [PIMG, CPC], [1, PH]])
            dst1 = AP(t1.tensor, t1.offset,
                      [[FREE, ROWS1], [PH, CPC], [1, PH]])
            nc.gpsimd.dma_start(out=dst1, in_=src1)

            # Stores: for each (kh, kw), 16 channels per DMA (the channel
            # dim is split in half so the (window-row, channel, col) walk
            # keeps the partition-crossing step on dim 0 and no dim merge
            # fires; 32-channel and 4-channel variants measured slower).
            # out row (c*9 + kh*3 + kw), col r*223.. = padded[kh+r, kw..kw+222];
            # window rows 0..n0-1 live in tile0 (partitions kh..127), the rest
            # in tile1 (partitions 0..n1-1).
            # Row counts 97/113/127 crash the SWDGE path on device
            # (NRT_EXEC_UNIT_UNRECOVERABLE, found empirically), so split
            # those transfers into known-good chunk sizes.
            def safe_rows(n):
                if n in (128, 126, 124, 121, 120, 112, 96, 95, 64, 63, 31, 15, 1):
                    return [n]
                for first in (112, 96, 64):
                    if 0 < n - first and (n - first) in (63, 31, 15, 1):
                        return [first, n - first]
                return [n - 15, 15]

            # Each store: (kh, kw, h, tile, chunk-start-row r, rows n).
            CH2 = CPC // 2
            work = []
            for kh in range(3):
                n0 = ROWS0 - kh
                n1 = OH - n0
                for kw in range(3):
                    for h in range(2):
                        r = 0
                        for n in safe_rows(n0):
                            work.append((kh, kw, h, 0, r, n))
                            r += n
                        for n in safe_rows(n1):
                            work.append((kh, kw, h, 1, r, n))
                            r += n

            def emit(eng, kh, kw, h, tl, r, n):
                co = h * CH2
                if tl == 0:
                    src = AP(t0.tensor,
                             t0.offset + (kh + r) * FREE + co * PH + kw,
                             [[FREE, n], [PH, CH2], [1, OH]])
                else:
                    src = AP(t1.tensor,
                             t1.offset + (r - (ROWS0 - kh)) * FREE + co * PH + kw,
                             [[FREE, n], [PH, CH2], [1, OH]])
                dst = AP(ob.tensor,
                         ob.offset + (co * 9 + kh * 3 + kw) * OSZ + r * OH,
                         [[OH, n], [9 * OSZ, CH2], [1, OH]])
                eng.dma_start(out=dst, in_=src)

            # Tiny split-remainder chunks go to the (otherwise idle) HWDGE
            # queues; the big stores stay on the fast SWDGE model queue,
            # ordered tile0-first so the queue never stalls on load1.
            small = [w for w in work if w[5] <= 15]
            big = [w for w in work if w[5] > 15]
            for i, (kh, kw, h, tl, r, n) in enumerate(small):
                emit(nc.sync if i % 2 == 0 else nc.scalar, kh, kw, h, tl, r, n)
            for kh, kw, h, tl, r, n in sorted(big, key=lambda w: w[3]):
                emit(nc.gpsimd, kh, kw, h, tl, r, n)
    return nc


def _split_multi_waits(nc: bass.Bass) -> None:
    """Walrus allows only one sync-wait command per instruction (the
    kernel-tail drain ends up with one per DMA-completion sem lane).
    Hoist all but the last wait onto fresh single-wait NOPs inserted
    just before the instruction on the same engine — semantically
    identical (the engine blocks on each wait in turn)."""
    from bass_rust import SyncInfo

    k = 0
    for fn in nc.m.functions:
        for blk in fn.blocks:
            insts = blk.instructions
            for idx in range(len(insts) - 1, -1, -1):
                inst = insts[idx]
                si = inst.sync_info
                if si is None or len(si.on_wait) <= 1:
                    continue
                waits = list(si.on_wait)
                for w in waits[:-1]:
                    nop = mybir.InstNoOp(name=f"WSPLIT-{k}")
                    k += 1
                    nop.engine = inst.engine
                    nop.sync_info = SyncInfo(on_wait=[w], on_update=[])
                    insts.insert(idx, nop)
                si.on_wait = [waits[-1]]
                inst.sync_info = si


def get_nc() -> bass.Bass:
    if "nc" not in _NC_CACHE:
        nc = build_nc()
        _split_multi_waits(nc)
        _NC_CACHE["nc"] = nc
    return _NC_CACHE["nc"]


def make_in_maps(x: np.ndarray) -> list[dict]:
    x = np.asarray(x, dtype=np.float32)
    xp = np.pad(x, ((0, 0), (0, 0), (1, 1), (1, 1)))
    maps = []
    for core in range(N_CORES):
        b, half = divmod(core, 2)
        maps.append({"xp": np.ascontiguousarray(xp[b, half * CPC:(half + 1) * CPC])})
    return maps


def gather_out(results: list[dict]) -> np.ndarray:
    out = np.empty((B, C * 9, OSZ), dtype=np.float32)
    for core in range(N_CORES):
        b, half = divmod(core, 2)
        out[b, half * NROW:(half + 1) * NROW] = results[core]["out"]
    return out


def kernel(**inputs) -> np.ndarray:
    x = inputs["x"]
    nc = get_nc()
    res = run_bass_kernel_spmd(nc, make_in_maps(x), list(range(N_CORES)))
    return gather_out(res.results)



# revision 2
# speedup vs baseline: 1.1030x; 1.1030x over previous
"""Unfold/im2col kernel for Trainium2 (Bass/Tile), 8-core data parallel, v2.

Problem: x [4, 64, 224, 224] f32 -> out [4, 576, 49729] f32 where
out[b, (c*3+kh)*3+kw, oh*223+ow] = pad(x,1)[b, c, oh+kh, ow+kw]
(3x3 kernel, pad 1, stride 1, dilation 1, oh=ow=223).

Sharding: 8 cores = (batch 4) x (channel half 2); each core handles 32
channels -> [288, 49729].

v2 strategy (vs v1's pure-DMA kernel at ~330 us):
1. bf16 wire format. Tolerance is rel_err < 2e-2; bf16 keeps f32's
   exponent range so per-element relative error <= 2^-9 ~= 2e-3. The
   host casts the padded input to bf16, the device stores bf16, the
   host upcasts on gather. Halves HBM traffic: 64 MB -> 32 MB/core.
2. Big store descriptors. v1 stored straight from a rows-on-partitions
   layout, which caps every DMA descriptor at 223 elems (892 B) and
   descriptor processing limits SDMA to ~200 GB/s. v2 instead uses the
   idle compute engines (DVE / ACT / GpSimd) to build each (kh, kw)
   output slice in SBUF in its final layout, so each store descriptor
   is a full 12488-elem (25 KB) contiguous run per partition and DMA
   runs at the ~358 GB/s HBM-per-core roofline.

Layout: partition p = j*32 + c for row-block j in [0,4), channel c in
[0,32). in_tile partition p holds padded rows [56j, 56j+58) x 226 cols.
For each (kh, kw), a strided compute copy compacts 56 rows x 223 cols
(shifted by kh, kw) into a dense 223-stride out buffer; two HWDGE DMAs
(j in {0,1,2}: 56 rows each; j=3: 55 rows) store it. Copies round-robin
over DVE/ACT/GpSimd so they hide under the stores.
"""

from contextlib import ExitStack

import ml_dtypes
import numpy as np

import concourse.bass as bass
import concourse.tile as tile
from concourse import mybir
from concourse.ap import AP
from concourse.bass_utils import run_bass_kernel_spmd

B, C, IH, IW = 4, 64, 224, 224
N_CORES = 8
CPC = C // 2          # channels per core: 32
PH = IH + 2           # padded height/width: 226
OH = IH - 1           # output spatial: 223
OSZ = OH * OH         # 49729
NROW = CPC * 9        # 288 output rows per core
PIMG = PH * PH        # padded image elements: 51076

NJ = 4                # row-blocks across partitions
RPB = 56              # output rows per block (last block uses 55)
TR = 58               # padded image rows held per partition
PF = TR * PH          # in-tile free elems per partition: 13108
OF = RPB * 224        # out-buf free elems per partition: 12544 (12488 used)
NB = RPB * OH         # full-block chunk elems: 12488
OSZP = NJ * NB        # padded DRAM row length: 49952 (= OSZ + 223 pad)

DT = mybir.dt.bfloat16
NPDT = ml_dtypes.bfloat16

_NC_CACHE = {}


def build_nc() -> bass.Bass:
    nc = bass.Bass()
    # Output rows padded 49729 -> 49952 so every (kh, kw) store is ONE
    # uniform 128-partition DMA (even engine spread); the 223-elem row
    # tail catches the j=3 block's garbage row and is sliced off on the
    # host.
    x = nc.declare_dram_parameter("xp", [CPC, PH, PH], DT, isOutput=False)
    out = nc.declare_dram_parameter("out", [NROW, OSZP], DT, isOutput=True)
    xb = x[:, :, :]
    ob = out[:, :]

    with tile.TileContext(nc) as tc:
        with ExitStack() as ctx:
            pool = ctx.enter_context(tc.tile_pool(name="img", bufs=1))
            it = pool.tile([128, PF], DT, name="it", tag="it")[:, :]
            NBUF = 5
            obufs = [
                pool.tile([128, OF], DT, name=f"ob{i}", tag=f"ob{i}")[:, :]
                for i in range(NBUF)
            ]

            # Load: partition (j*32 + c) <- xp[c, 56j : 56j+58, :].
            # 26 KB contiguous per partition; consecutive j blocks
            # re-read their 2-row overlap. SWDGE (gpsimd): HWDGE rings
            # only engage ~3 SDMA engines (~72 GB/s measured); SWDGE
            # sprays all 16. One DMA per j block: the AP normalizer
            # splits work across SDMA queues by the OUTERMOST dim, so a
            # single load with outer dim [j, 4] lands on only 4 engines
            # (measured 62 us); four 32-partition loads spray all 16.
            for j in range(NJ):
                nc.gpsimd.dma_start(
                    out=AP(it.tensor, it.offset + j * CPC * PF, [[PF, CPC], [1, PF]]),
                    in_=AP(xb.tensor, xb.offset + j * RPB * PH, [[PIMG, CPC], [1, PF]]),
                )

            def copy(eng, dst, src):
                if eng is nc.scalar:
                    eng.copy(out=dst, in_=src)
                else:
                    eng.tensor_copy(out=dst, in_=src)

            # DVE copy measured 3.4 us (4x perf mode), ACT 10.7 us, GpSimd
            # busy with SWDGE descriptor gen -> DVE gets 6, ACT 3.
            for K in range(9):
                kh, kw = divmod(K, 3)
                o = obufs[K % NBUF]
                eng = nc.scalar if K % 3 == 0 else nc.vector
                # Compact copy: o[p][r*223 + w] = it[p][(r+kh)*226 + (w+kw)],
                # r in [0,56), w in [0,223). Innermost 222 (even -> DVE 2x
                # perf mode) + a 1-col tail.
                copy(
                    eng,
                    AP(o.tensor, o.offset, [[OF, 128], [OH, RPB], [1, 222]]),
                    AP(it.tensor, it.offset + kh * PH + kw,
                       [[PF, 128], [PH, RPB], [1, 222]]),
                )
                copy(
                    eng,
                    AP(o.tensor, o.offset + 222, [[OF, 128], [OH, RPB], [1, 1]]),
                    AP(it.tensor, it.offset + kh * PH + kw + 222,
                       [[PF, 128], [PH, RPB], [1, 1]]),
                )
                # Stores: out row (c*9 + K), cols [12488j, ...). One DMA for
                # j in {0,1,2} (full 56-row chunks), one for j=3 (55 rows).
                nc.gpsimd.dma_start(
                    out=AP(ob.tensor, ob.offset + K * OSZP,
                           [[NB, NJ], [9 * OSZP, CPC], [1, NB]]),
                    in_=AP(o.tensor, o.offset, [[OF, 128], [1, NB]]),
                )
    return nc


def _split_multi_waits(nc: bass.Bass) -> None:
    """Walrus allows only one sync-wait command per instruction (the
    kernel-tail drain ends up with one per DMA-completion sem lane).
    Hoist all but the last wait onto fresh single-wait NOPs inserted
    just before the instruction on the same engine — semantically
    identical (the engine blocks on each wait in turn)."""
    from bass_rust import SyncInfo

    k = 0
    for fn in nc.m.functions:
        for blk in fn.blocks:
            insts = blk.instructions
            for idx in range(len(insts) - 1, -1, -1):
                inst = insts[idx]
                si = inst.sync_info
                if si is None or len(si.on_wait) <= 1:
                    continue
                waits = list(si.on_wait)
                for w in waits[:-1]:
                    nop = mybir.InstNoOp(name=f"WSPLIT-{k}")
                    k += 1
                    nop.engine = inst.engine
                    nop.sync_info = SyncInfo(on_wait=[w], on_update=[])
                    insts.insert(idx, nop)
                si.on_wait = [waits[-1]]
                inst.sync_info = si


def get_nc() -> bass.Bass:
    if "nc" not in _NC_CACHE:
        nc = build_nc()
        _split_multi_waits(nc)
        _NC_CACHE["nc"] = nc
    return _NC_CACHE["nc"]


def make_in_maps(x: np.ndarray) -> list[dict]:
    x = np.asarray(x, dtype=np.float32)
    xp = np.pad(x, ((0, 0), (0, 0), (1, 1), (1, 1))).astype(NPDT)
    maps = []
    for core in range(N_CORES):
        b, half = divmod(core, 2)
        maps.append({"xp": np.ascontiguousarray(xp[b, half * CPC:(half + 1) * CPC])})
    return maps


def gather_out(results: list[dict]) -> np.ndarray:
    out = np.empty((B, C * 9, OSZ), dtype=np.float32)
    for core in range(N_CORES):
        b, half = divmod(core, 2)
        out[b, half * NROW:(half + 1) * NROW] = (
            results[core]["out"][:, :OSZ].astype(np.float32)
        )
    return out


def kernel(**inputs) -> np.ndarray:
    x = inputs["x"]
    nc = get_nc()
    res = run_bass_kernel_spmd(nc, make_in_maps(x), list(range(N_CORES)))
    return gather_out(res.results)
